# revision 34
# baseline (speedup 1.0000x reference)
"""Causal self-attention (B=4, T=2048, C=1024, H=16) on 8 TRN2 NeuronCores.

Sharding: core = (batch, head-group) — data parallel over the 4 batches,
tensor parallel over 2 groups of 8 heads (Megatron-style column/row split of
the qkv / out projections).  Each core computes a [T, C] partial of the out
projection for its head group; the host sums the two partials per batch and
adds b_out, so no device collectives are needed.

All matmul operands are bf16 (PE streams 1 col/cycle regardless of dtype,
so bf16 costs nothing on the PE but halves SBUF/DMA traffic, enables FWL
weight loads, and lets an S^T block-pair share one PSUM bank).  PSUM
accumulators stay fp32 except the S^T logits, which are written bf16 so the
even/odd head pair packs into a single bank and one ScalarE Exp covers both
(halving ACT instruction overhead, the phase-B bottleneck).

Device kernel, per tq-slab of 512:
  A(s) q^T,k^T = (W_qk chunk)^T @ x^T slab; v = x @ W_v (+bias via K=1 ones
       matmul).  PSUM evictions on DVE (tensor_scalar: scale+bias).
  B(h,s) S^T pair = k^T-block @ q^T-slab (K=64; odd heads in partitions
       64-127 so even/odd pairs overlap in distinct PE row groups), one
       merged Exp per pair on ScalarE, causal 0/1 mask post-exp on GpSimd
       for the diagonal subtiles, then P@V with lhsT=[v | 1] so the softmax
       denominator accumulates for free as PSUM row D.  Blocks run in
       DESCENDING tk order so the masked diagonal work is off the slab tail
       (has_written handles the ragged first write).  Normalization without
       any DRAM bounce: d -> SBUF (DVE), K=1 ones-matmul broadcasts d into
       partitions 64-127 of the same PSUM bank, reciprocal_approx_fast +
       tensor_mul write the normalized y^T straight into yT_sb.
  C(s) out partial = y^T chunks @ W_out chunks.

To keep the PE instruction queue free of multi-us stalls (which re-trip the
HAM clock throttle and halve the PE clock), A(s+1) and C(s-1) are emitted as
work units INTERLEAVED into B(s)'s block loop, so the statically scheduled
PE stream always has projection matmuls to chew on while ScalarE works
through the Exp backlog.
"""

import os
import sys
from contextlib import ExitStack

import numpy as np
import ml_dtypes

for _p in ("/opt/trn_rl_repo", "/root/.axon_site/_ro/trn_rl_repo"):
    if os.path.isdir(_p) and _p not in sys.path:
        sys.path.append(_p)

import concourse.bacc as bacc
import concourse.bass as bass
import concourse.tile as tile
from concourse import mybir
from concourse.bass_utils import run_bass_kernel_spmd
from concourse.masks import make_upper_triangular

AF = mybir.ActivationFunctionType
ALU = mybir.AluOpType
F32 = mybir.dt.float32
F32R = mybir.dt.float32r
BF16 = mybir.dt.bfloat16

P = 128
SLAB = 512

B, T, C, H, D = 4, 2048, 1024, 16, 64
N_CORES = 8
N_GROUPS = 2          # head groups (tensor-parallel degree per batch)
HL = H // N_GROUPS    # heads per core
CL = HL * D           # local qkv width


def _build_nc(loop_reps=None, debug_dump=False):
    NCK = C // P
    MQK = 2 * CL // P
    MQ = MQK // 2
    TT = T // P
    NS = T // SLAB
    YC = CL // P
    W_OUT = min(SLAB, C)
    NOUT = C // W_OUT
    scale = 1.0 / np.sqrt(D)

    nc = bacc.Bacc("TRN2", target_bir_lowering=False, debug=False,
                   num_devices=N_CORES)
    xT = nc.dram_tensor("xT", [C, T], BF16, kind="ExternalInput")
    wqk = nc.dram_tensor("wqk", [C, 2 * CL], BF16, kind="ExternalInput")
    wv = nc.dram_tensor("wv", [C, CL], BF16, kind="ExternalInput")
    wout = nc.dram_tensor("wout", [CL, C], BF16, kind="ExternalInput")
    bqk = nc.dram_tensor("bqk", [P, MQK], F32, kind="ExternalInput")
    bv = nc.dram_tensor("bv", [1, CL], BF16, kind="ExternalInput")
    outp = nc.dram_tensor("outp", [T, C], BF16, kind="ExternalOutput")
    if debug_dump:
        dbg_d = nc.dram_tensor("dbg_d", [8, SLAB], F32, kind="ExternalOutput")
        dbg_b = nc.dram_tensor("dbg_b", [8, SLAB], F32, kind="ExternalOutput")
        dbg_y = nc.dram_tensor("dbg_y", [P, 4 * SLAB], F32,
                               kind="ExternalOutput")

    with tile.TileContext(nc) as tc, ExitStack() as ctx:
        pool = lambda name, bufs, **kw: ctx.enter_context(
            tc.tile_pool(name=name, bufs=bufs, **kw))

        const = pool("const", 1)
        kp = pool("kp", 1)
        vp = pool("vp", 1)
        wqkp = pool("wqkp", 1)
        wvp = pool("wvp", 1)
        woutp = pool("woutp", 1)
        xtp = pool("xt", 2)
        qp = pool("qp", 2)
        yTp = pool("yTp", 2)
        expp = pool("expp", 3)
        dp = pool("dp", 2)
        binvp = pool("binvp", 2)
        otp = pool("ot", 2)
        psPO = pool("psPO", 2, space="PSUM")
        psS = pool("psS", 2, space="PSUM")
        psY = pool("psY", 1, space="PSUM")

        k_sb = kp.tile([P, MQ, T], BF16)
        v_sb = vp.tile([P, TT, HL, D + 1], BF16)
        wqk_sb = wqkp.tile([P, NCK, 2 * CL], BF16)
        wv_sb = wvp.tile([P, NCK, CL], BF16)
        wout_sb = woutp.tile([P, YC, C], BF16)
        bqk_sb = const.tile([P, MQK], F32)
        bv_sb = const.tile([1, CL], BF16)
        mask_f = const.tile([P, P], F32)
        mask01 = const.tile([P, P], BF16)
        onescr = const.tile([P, TT * HL], F32)
        ones64b = const.tile([1, 64], BF16)

        warm_f = const.tile([P, SLAB], F32)
        cwarm = const.tile([P, SLAB], BF16)
        wscr = const.tile([1, 1], F32)

        def emit_init_dmas():
            # wqk first so the first q/k m-tiles can start ASAP; the
            # v / out-projection weights follow.
            for c in range(NCK):
                nc.sync.dma_start(out=wqk_sb[:, c, :],
                                  in_=wqk[c * P:(c + 1) * P, :])
            nc.sync.dma_start(out=bqk_sb[:, :], in_=bqk[:, :])
            nc.sync.dma_start(out=bv_sb[:, :], in_=bv[:, :])
            for c in range(NCK):
                nc.sync.dma_start(out=wv_sb[:, c, :],
                                  in_=wv[c * P:(c + 1) * P, :])
            for c in range(YC):
                nc.sync.dma_start(out=wout_sb[:, c, :],
                                  in_=wout[c * P:(c + 1) * P, :])

        # mask01[p, f] = 1 if f >= p else 0  (S^T visibility: tq >= tk).
        make_upper_triangular(nc, mask_f[:, :], val=1.0, diag=True)
        nc.vector.tensor_copy(mask01[:, :], mask_f[:, :])
        nc.vector.memset(onescr[:, :], 1.0)
        nc.vector.tensor_copy(
            v_sb[:, :, :, D],
            onescr[:, :].rearrange("p (t h) -> p t h", h=HL))
        nc.vector.tensor_copy(ones64b[0:1, :], onescr[0:1, 0:64])
        nc.vector.memset(warm_f[:, :], 1.0)
        nc.vector.tensor_copy(cwarm[:, :], warm_f[:, :])
        ones1 = v_sb[0:1, :, :, D].rearrange("u t h -> u (t h)")

        def emit_warmup(n_mm):
            # Keep the PE busy on const data while the weight/x DMAs land,
            # so the HAM clock gate is at 8/8 when real matmuls start.
            # Full-array K=128 matmuls — HAM watches PE activity, so thin
            # matmuls don't register as busy.
            ps_w = psS.tile([P, 2, SLAB], F32, tag="s")
            for _ in range(n_mm):
                nc.tensor.matmul(ps_w[:, 0, :], mask01[:, :],
                                 cwarm[:, :], start=True, stop=True)
            nc.vector.tensor_copy(wscr[0:1, 0:1], ps_w[0:1, 0, 0:1])

        def emit_xt_load(s):
            t0 = s * SLAB
            xt = xtp.tile([P, NCK, SLAB], BF16)
            for c in range(NCK):
                nc.sync.dma_start(out=xt[:, c, :],
                                  in_=xT[c * P:(c + 1) * P, t0:t0 + SLAB])
            return xt

        def make_a_units(s, xt):
            """Projection work units for slab s, split to <=~850ns of PE
            work each so an interleaved unit never starves ScalarE."""
            t0 = s * SLAB
            q_sb = qp.tile([P, MQ, SLAB], BF16)
            HC = NCK // 2

            def qk_units(m):
                cell = {}

                def run_a():
                    ps = psPO.tile([P, SLAB], F32, tag="ps")
                    cell["ps"] = ps
                    for c in range(HC):
                        nc.tensor.matmul(
                            ps[:, :],
                            wqk_sb[:, c, m * P:(m + 1) * P],
                            xt[:, c, :],
                            start=(c == 0), stop=False)

                def run_b():
                    ps = cell["ps"]
                    for c in range(HC, NCK):
                        nc.tensor.matmul(
                            ps[:, :],
                            wqk_sb[:, c, m * P:(m + 1) * P],
                            xt[:, c, :],
                            start=False, stop=(c == NCK - 1))
                    dst = (q_sb[:, m, :] if m < MQ
                           else k_sb[:, m - MQ, t0:t0 + SLAB])
                    sc = scale if m < MQ else 1.0
                    nc.vector.tensor_scalar(
                        dst, ps[:, :], sc, bqk_sb[:, m:m + 1],
                        op0=ALU.mult, op1=ALU.add)

                return [run_a, run_b]

            def v_units(sub):
                cell = {}

                def run_a():
                    ps = psPO.tile([P, CL], F32, tag="ps")
                    cell["ps"] = ps
                    for c in range(HC):
                        nc.tensor.matmul(
                            ps[:, :],
                            xt[:, c, sub * P:(sub + 1) * P],
                            wv_sb[:, c, :],
                            start=(c == 0), stop=False)

                def run_b():
                    tt = s * (SLAB // P) + sub
                    ps = cell["ps"]
                    for c in range(HC, NCK):
                        nc.tensor.matmul(
                            ps[:, :],
                            xt[:, c, sub * P:(sub + 1) * P],
                            wv_sb[:, c, :],
                            start=False, stop=False)
                    nc.tensor.matmul(
                        ps[:, :], ones1[:, :],
                        bv_sb[0:1, :], start=False, stop=True)
                    nc.vector.tensor_copy(
                        v_sb[:, tt, :, 0:D],
                        ps[:, :].rearrange("p (h d) -> p h d", d=D))

                return [run_a, run_b]

            # Dependency-friendly order: (q_m, k_m) pairs so B of this slab
            # can start per-head-pair as soon as its q/k/v tiles land.
            units = qk_units(0) + qk_units(MQ)
            for sub in range(SLAB // P):
                units += v_units(sub)
            for m in range(1, MQ):
                units += qk_units(m) + qk_units(MQ + m)
            return q_sb, units

        def make_c_units(s, yT_sb):
            """Out-projection work units for slab s (inputs: yT tile of s)."""
            t0 = s * SLAB

            def c_units(sub, n, on_scalar):
                cell = {}
                n0 = n * W_OUT

                def run_a():
                    ps = psPO.tile([P, W_OUT], F32, tag="ps")
                    cell["ps"] = ps
                    for c in range(YC // 2):
                        nc.tensor.matmul(
                            ps[:, :],
                            yT_sb[:, c, sub * P:(sub + 1) * P],
                            wout_sb[:, c, n0:n0 + W_OUT],
                            start=(c == 0), stop=False)

                def run_b():
                    ps = cell["ps"]
                    for c in range(YC // 2, YC):
                        nc.tensor.matmul(
                            ps[:, :],
                            yT_sb[:, c, sub * P:(sub + 1) * P],
                            wout_sb[:, c, n0:n0 + W_OUT],
                            start=False, stop=(c == YC - 1))
                    ot = otp.tile([P, W_OUT], BF16)
                    # alternate evictions between ScalarE (Copy: no table)
                    # and DVE so neither engine serializes the C stream
                    if on_scalar:
                        nc.scalar.copy(ot[:, :], ps[:, :])
                    else:
                        nc.vector.tensor_copy(ot[:, :], ps[:, :])
                    nc.sync.dma_start(
                        out=outp[t0 + sub * P:t0 + (sub + 1) * P,
                                 n0:n0 + W_OUT],
                        in_=ot[:, :])

                return [run_a, run_b]

            units = []
            for sub in range(SLAB // P):
                for n in range(NOUT):
                    units += c_units(sub, n, (sub * NOUT + n) % 2 == 0)
            return units

        def emit_b(s, q_sb, units):
            """Attention for slab s; drains `units` into the block loop.

            The S^T matmul pair for block b+1 is emitted BEFORE the P@V
            pair of block b, so the next Exp's input is at the head of
            the PE queue and ScalarE stays saturated."""
            t0 = s * SLAB
            nblk = (s + 1) * SLAB // P
            nsteps = 4 * nblk
            yT_sb = yTp.tile([P, YC, SLAB], BF16)
            ucur = 0
            ucredit = 0.0
            upd = len(units) / nsteps

            def emit_s_pair(hp, b):
                tk0 = b * P
                vis = max(0, tk0 - t0)
                ps = psS.tile([P, 2, SLAB], F32, tag="s")
                for i in range(2):
                    row0 = i * 64
                    nc.tensor.matmul(
                        ps[:, i, vis:SLAB],
                        k_sb[row0:row0 + 64, hp, tk0:tk0 + P],
                        q_sb[row0:row0 + 64, hp, vis:SLAB],
                        start=True, stop=True,
                        tile_position=(row0, 0))
                return ps

            blocks = list(range(nblk - 1, -1, -1))
            ps_cur = emit_s_pair(0, blocks[0])
            for hp in range(HL // 2):
                py0 = psY.tile([P, SLAB], F32, tag="py0")
                py1 = psY.tile([P, SLAB], F32, tag="py1")
                pys = (py0, py1)
                for bi, b in enumerate(blocks):
                    tk0 = b * P
                    off = tk0 - t0
                    vis = max(0, off)
                    ep = expp.tile([P, 2, SLAB], BF16)
                    nc.scalar.activation(ep[:, :, vis:SLAB],
                                         ps_cur[:, :, vis:SLAB], AF.Exp)
                    # hoist the next S pair (next block, or the next
                    # head-pair's first block) ahead of the P@V and the
                    # eviction chain in the PE stream, so ScalarE's next
                    # Exp input is never behind them
                    if bi + 1 < nblk:
                        ps_cur = emit_s_pair(hp, blocks[bi + 1])
                    elif hp + 1 < HL // 2:
                        ps_cur = emit_s_pair(hp + 1, blocks[0])
                    else:
                        ps_cur = None
                    if off >= 0:
                        for i in range(2):
                            nc.gpsimd.tensor_mul(
                                ep[:, i, off:off + P], ep[:, i, off:off + P],
                                mask01[:, :])
                    for i in range(2):
                        nc.tensor.matmul(
                            pys[i][0:D + 1, vis:SLAB],
                            v_sb[:, b, 2 * hp + i, 0:D + 1],
                            ep[:, i, vis:SLAB],
                            start=(bi == 0), stop=(b == 0))
                    ucredit += upd
                    while ucur < len(units) and ucur < int(ucredit):
                        units[ucur]()
                        ucur += 1
                # Free the py banks ASAP (they gate the next head-pair's
                # P@V through the single-buffered psY pool): evict the raw
                # numerator + denominator first, then normalize in place
                # off the critical path (reciprocal -> K=1 ones-matmul
                # broadcast into a borrowed psPO bank -> in-place multiply).
                d_sbs = []
                for i in range(2):
                    row0 = i * 64
                    d_sb = dp.tile([1, SLAB], F32, tag=f"d{i}")
                    nc.vector.tensor_copy(d_sb[0:1, :], pys[i][D:D + 1, :])
                    nc.vector.tensor_copy(
                        yT_sb[row0:row0 + 64, hp, :], pys[i][0:D, :])
                    d_sbs.append(d_sb)
                for i in range(2):
                    row0 = i * 64
                    rin = dp.tile([1, SLAB], F32, tag=f"r{i}")
                    nc.vector.reciprocal_approx_fast(rin[0:1, :],
                                                     d_sbs[i][0:1, :])
                    rb = dp.tile([1, SLAB], BF16, tag=f"rb{i}")
                    nc.vector.tensor_copy(rb[0:1, :], rin[0:1, :])
                    bc = psPO.tile([P, W_OUT], F32, tag="ps")
                    nc.tensor.matmul(
                        bc[0:64, :], ones64b[0:1, :], rb[0:1, :],
                        start=True, stop=True)
                    nc.vector.tensor_mul(
                        yT_sb[row0:row0 + 64, hp, :],
                        yT_sb[row0:row0 + 64, hp, :], bc[0:64, :])
                    if debug_dump and s == 0:
                        di = 2 * hp + i
                        nc.sync.dma_start(out=dbg_d[di:di + 1, :],
                                          in_=d_sbs[i][0:1, :])
                        nc.sync.dma_start(out=dbg_b[di:di + 1, :],
                                          in_=rin[0:1, :])
            while ucur < len(units):
                units[ucur]()
                ucur += 1
            if debug_dump and s == 0:
                ysc = binvp.tile([P, 4 * SLAB], F32, tag="ydbg")
                nc.vector.tensor_copy(
                    ysc[:, :].rearrange("p (c t) -> p c t", c=4),
                    yT_sb[:, :, :])
                nc.sync.dma_start(out=dbg_y[:, :], in_=ysc[:, :])
            return yT_sb

        def weave(l1, l2):
            out = []
            i = j = 0
            n1, n2 = len(l1), len(l2)
            while i < n1 or j < n2:
                if j >= n2 or (i < n1 and i * (n2 + 1) <= j * (n1 + 1)):
                    out.append(l1[i])
                    i += 1
                else:
                    out.append(l2[j])
                    j += 1
            return out

        def body():
            NSL = T // SLAB
            xt = emit_xt_load(0)
            emit_init_dmas()
            emit_warmup(32)
            q_sb, a_units = make_a_units(0, xt)
            # run just enough of A(0) for head-pair 0 (q0,k0,v*), push the
            # rest into B(0)'s interleave list
            for u in a_units[:12]:
                u()
            carry = a_units[12:]
            yT_prev = None
            for s in range(NSL):
                if s + 1 < NSL:
                    xt = emit_xt_load(s + 1)
                    q_next, a_units = make_a_units(s + 1, xt)
                else:
                    q_next = None
                    a_units = []
                c_units = (make_c_units(s - 1, yT_prev)
                           if yT_prev is not None else [])
                units = carry + weave(a_units, c_units)
                carry = []
                yT_prev = emit_b(s, q_sb, units)
                q_sb = q_next
            for u in make_c_units(NSL - 1, yT_prev):
                u()

        if loop_reps is None:
            body()
        else:
            with tc.For_i(0, loop_reps, 1):
                body()

    nc.compile()
    return nc


_NC_CACHE = None


def _get_nc():
    global _NC_CACHE
    if _NC_CACHE is None:
        _NC_CACHE = _build_nc()
    return _NC_CACHE


def make_in_maps(x, W_qkv, b_qkv, W_out):
    scale = 1.0 / np.sqrt(D)
    MQK = 2 * CL // P
    bf = ml_dtypes.bfloat16
    in_maps = []
    for core in range(N_CORES):
        b, hg = divmod(core, N_GROUPS)
        qs = slice(hg * CL, (hg + 1) * CL)
        ks = slice(C + hg * CL, C + (hg + 1) * CL)
        vs = slice(2 * C + hg * CL, 2 * C + (hg + 1) * CL)
        bqk_cat = np.concatenate([b_qkv[qs] * scale, b_qkv[ks]])
        in_maps.append({
            "xT": np.ascontiguousarray(x[b].T.astype(bf)),
            "wqk": np.ascontiguousarray(
                np.concatenate([W_qkv[:, qs], W_qkv[:, ks]],
                               axis=1).astype(bf)),
            "wv": np.ascontiguousarray(W_qkv[:, vs].astype(bf)),
            "wout": np.ascontiguousarray(
                W_out[hg * CL:(hg + 1) * CL, :].astype(bf)),
            "bqk": np.ascontiguousarray(bqk_cat.reshape(MQK, P).T
                                        .astype(np.float32)),
            "bv": np.ascontiguousarray(b_qkv[vs].reshape(1, CL).astype(bf)),
        })
    return in_maps


def kernel(x, W_qkv, b_qkv, W_out, b_out):
    x = np.asarray(x, dtype=np.float32)
    W_qkv = np.asarray(W_qkv, dtype=np.float32)
    b_qkv = np.asarray(b_qkv, dtype=np.float32)
    W_out = np.asarray(W_out, dtype=np.float32)
    b_out = np.asarray(b_out, dtype=np.float32)

    nc = _get_nc()
    in_maps = make_in_maps(x, W_qkv, b_qkv, W_out)
    res = run_bass_kernel_spmd(nc, in_maps, core_ids=list(range(N_CORES)))

    out = np.empty((B, T, C), dtype=np.float32)
    for b in range(B):
        out[b] = (res.results[N_GROUPS * b]["outp"].astype(np.float32)
                  + res.results[N_GROUPS * b + 1]["outp"].astype(np.float32)
                  + b_out)
    return out
